# revision 35
# baseline (speedup 1.0000x reference)
"""Multi-head cross-attention (B=4, Sq=Skv=2048, E=1024, H=16, D=64) on 8
Trainium2 NeuronCores.

Sharding: core c -> (batch b = c//2, head-group g = c%2 of 8 heads).
Each core computes, for its batch and its 8 heads:
    qh = q @ wq.T (per head), kh/vh likewise (kv),
    scoresT[t,s] = kh . qh  (transposed orientation, t on partitions),
    attnT = exp(scoresT)    (no max subtraction; scores ~ N(0,1)),
    ctxT[d,s]  = sum_t vh[t,d] attnT[t,s]       (PSUM accumulate over t),
    denom[s]   = sum_t attnT[t,s]               (65th "ones" column of vh),
    ctxN       = ctxT * (1/denom),
    outT[e,s] += woT[hd,e].T @ ctxN[hd,s]       (partial W_O, this core's heads).
Host sums the two head-group partials per batch, transposes, adds bias terms.

Bias handling: bq==0 guaranteed by the problem spec (fill=zeros); bk is
mathematically a no-op for softmax (adds a per-query constant to scores);
bv folds to +bv after normalization, handled on host via wo @ bv; bo added
on host.

dtype: bf16 on the tensor engine with fp32 PSUM accumulation.
"""

import sys
import types

import numpy as np


def _ensure_paths():
    try:
        import concourse.bass  # noqa: F401
    except ImportError:
        for p in ("/opt/trn_rl_repo", "/root/.axon_site/_ro/trn_rl_repo"):
            if p not in sys.path:
                sys.path.append(p)


def _install_ntff_hook():
    """Register the axon NTFF profiling hook if the image's antenv lacks it.
    Only needed when tracing (BASS_TRACE=1); harmless otherwise."""
    try:
        from antenv.axon_hooks import get_axon_ntff_profile_hook  # noqa: F401

        return
    except ImportError:
        pass
    try:
        import antenv
        from trn_agent_boot.trn_boot import _ntff_profile_via_ctypes

        mod = types.ModuleType("antenv.axon_hooks")
        _h = [None]
        mod.set_axon_ntff_profile_hook = lambda h: _h.__setitem__(0, h)
        mod.get_axon_ntff_profile_hook = lambda: _h[0]
        sys.modules["antenv.axon_hooks"] = mod
        antenv.axon_hooks = mod
        mod.set_axon_ntff_profile_hook(
            _ntff_profile_via_ctypes("/opt/axon/libaxon_pjrt.so")
        )
    except Exception:
        pass


_ensure_paths()
_install_ntff_hook()

import ml_dtypes  # noqa: E402
from contextlib import ExitStack  # noqa: E402

import concourse.bass as bass  # noqa: E402
import concourse.tile as tile  # noqa: E402
from concourse import bacc, mybir  # noqa: E402
from concourse.bass_utils import run_bass_kernel_spmd  # noqa: E402

BF16 = mybir.dt.bfloat16
F32 = mybir.dt.float32
bf16 = ml_dtypes.bfloat16

B, S, E, H, D = 4, 2048, 1024, 16, 64
NPAIR = 4          # head pairs per core (8 heads)
SC, NSC = 512, 4   # s-chunk
TB, NTB = 128, 16  # t-block
EXP = mybir.ActivationFunctionType.Exp


def _emit(tc, dram):
    nc = tc.nc
    qT_d, kvT_d, wq_d, wk_d, wv_d, woT_d, out_d = dram

    with ExitStack() as ctx:
        persist = ctx.enter_context(tc.tile_pool(name="persist", bufs=1))

        def ptile(shape, tag):
            return persist.tile(shape, BF16, tag=tag, name=tag)

        qhT = [ptile([128, S], f"qhT{p}") for p in range(NPAIR)]
        khT = [ptile([128, S], f"khT{p}") for p in range(NPAIR)]
        # vh1[h]: [t(128), NTB, 65]; col 64 = softmax-denominator ones column
        vh1 = [ptile([128, NTB, 65], f"vh1_{h}") for h in range(2 * NPAIR)]
        ctxN = [ptile([128, S], f"ctxN{p}") for p in range(NPAIR)]
        woT = [ptile([128, E], f"woT{p}") for p in range(NPAIR)]
        # qkv weights: one [128, NPAIR, D] tile each (one DMA submit each —
        # each dma_start costs ~650ns of Sync-engine issue time)
        wq_all = ptile([128, NPAIR, D], "wq_all")
        wk_all = ptile([128, NPAIR, D], "wk_all")
        wv_all = ptile([128, NPAIR, D], "wv_all")
        wq_sb = [wq_all[:, p, :] for p in range(NPAIR)]
        wk_sb = [wk_all[:, p, :] for p in range(NPAIR)]
        wv_sb = [wv_all[:, p, :] for p in range(NPAIR)]

        for (dst, src) in ((wq_all, wq_d), (wk_all, wk_d), (wv_all, wv_d)):
            nc.sync.dma_start(
                out=dst[:], in_=src.rearrange("(p i) e -> i p e", p=NPAIR))
        for h in range(2 * NPAIR):
            nc.vector.memset(vh1[h][:, :, 64:65], 1.0)

        inp = ctx.enter_context(tc.tile_pool(name="inp", bufs=4))
        drp = ctx.enter_context(tc.tile_pool(name="drp", bufs=4, space="DRAM"))
        attn_pool = ctx.enter_context(tc.tile_pool(name="attn", bufs=10))
        small = ctx.enter_context(tc.tile_pool(name="small", bufs=4))
        rbp = ctx.enter_context(tc.tile_pool(name="rbp", bufs=4))
        ctxu_pool = ctx.enter_context(tc.tile_pool(name="ctxu", bufs=4))
        ps_sc = ctx.enter_context(tc.tile_pool(name="ps_sc", bufs=2, space="PSUM"))
        ps_ctx = ctx.enter_context(tc.tile_pool(name="ps_ctx", bufs=2, space="PSUM"))
        proj_stack = ExitStack()
        ps_proj = proj_stack.enter_context(
            tc.tile_pool(name="ps_proj", bufs=1, space="PSUM"))
        ps_v = proj_stack.enter_context(
            tc.tile_pool(name="ps_v", bufs=1, space="PSUM"))

        # deferred PE work (ctx / W_O chunks) interleaved into later emission
        pending = []

        def drain(n):
            for _ in range(n):
                if pending:
                    pending.pop(0)()

        def proj_chunks(p):
            """Projection work for pair p as a list of small emit-callables
            (so pair p+1's projections interleave into pair p's attention)."""
            chunks = []
            state = {}

            def load_inputs():
                qT_t = inp.tile([128, S], BF16, tag="inp", name="qT_t")
                nc.sync.dma_start(out=qT_t[:], in_=qT_d[p * 128:(p + 1) * 128, :])
                kvT_t = inp.tile([128, S], BF16, tag="inp", name="kvT_t")
                nc.sync.dma_start(out=kvT_t[:], in_=kvT_d[p * 128:(p + 1) * 128, :])
                state["qT"], state["kv"] = qT_t, kvT_t

            chunks.append(load_inputs)

            def qk(which, sc):
                def go():
                    w_sb = wq_sb[p] if which == 0 else wk_sb[p]
                    src = state["qT"] if which == 0 else state["kv"]
                    dst = qhT[p] if which == 0 else khT[p]
                    ps = ps_proj.tile([128, SC], F32, tag="proj", name="ps")
                    cs = slice(sc * SC, (sc + 1) * SC)
                    nc.tensor.matmul(ps[0:64, :], w_sb[0:64, :],
                                     src[0:64, cs], start=True, stop=True)
                    nc.tensor.matmul(ps[64:128, :], w_sb[64:128, :],
                                     src[64:128, cs], start=True, stop=True)
                    nc.vector.tensor_copy(dst[:, cs], ps[:])
                return go

            def vproj(hl, tq):
                def go():
                    h = 2 * p + hl
                    hs = slice(hl * 64, (hl + 1) * 64)
                    psv = ps_v.tile([128, 4, D], F32, tag="v", name="psv")
                    for j in range(4):
                        tb = 4 * tq + j
                        nc.tensor.matmul(
                            psv[:, j, :],
                            state["kv"][hs, tb * TB:(tb + 1) * TB],
                            wv_sb[p][hs, :], start=True, stop=True)
                    nc.vector.tensor_copy(
                        vh1[h][:, 4 * tq:4 * tq + 4, 0:64], psv[:])
                return go

            # Order matters: Tile tracks deps only on already-emitted
            # instructions, so every chunk must be emitted before the first
            # instruction that reads its output. The consumer-aware order
            # below lets chunks drain lazily during the *previous* pair's
            # attention (or, for pair 0, during its own first s-chunk):
            #   q0,k0 first (first scores), then k1..k3 / v interleaved
            #   early (scores tb>=4, ctx matmuls), q1..q3 last (s-chunk>=1).
            chunks.append(qk(0, 0))
            chunks.append(qk(1, 0))
            chunks.append(vproj(0, 0))
            chunks.append(vproj(1, 0))
            chunks.append(qk(1, 1))
            chunks.append(vproj(0, 1))
            chunks.append(vproj(1, 1))
            chunks.append(qk(1, 2))
            chunks.append(vproj(0, 2))
            chunks.append(vproj(1, 2))
            chunks.append(qk(1, 3))
            chunks.append(vproj(0, 3))
            chunks.append(vproj(1, 3))
            chunks.append(qk(0, 1))
            chunks.append(qk(0, 2))
            chunks.append(qk(0, 3))
            return chunks

        def queue_norm(p, sc, ctx_tiles):
            # normalize ctx by the denominator row for both heads
            cs = slice(sc * SC, (sc + 1) * SC)
            for hl in range(2):
                ctx_ps = ctx_tiles[hl]

                def norm(p=p, hl=hl, ctx_ps=ctx_ps, cs=cs):
                    # Copy the whole [65,SC] PSUM tile to SBUF right away so
                    # the PSUM slot frees fast; the slow reciprocal then runs
                    # off the critical path.
                    cu = ctxu_pool.tile([65, SC], F32, tag="cu", name="cu")
                    nc.vector.tensor_copy(cu[:], ctx_ps[:])
                    # approx reciprocal (~3e-6 rel err). Quirks: must not be
                    # in-place, and needs a base-partition-0 range (a [64:65]
                    # slice returns garbage) — so run it over all 65 rows and
                    # use only row 64 (denominators; other rows are unused).
                    rp = small.tile([65, SC], F32, tag="r0", name="rp")
                    nc.vector.reciprocal_approx_fast(out=rp[:], in_=cu[:])
                    # partition-broadcast via DRAM bounce (SBUF sources
                    # require nonzero partition stride)
                    dr = drp.tile([1, SC], F32, tag="dr", name="dr")
                    nc.sync.dma_start(out=dr[:], in_=rp[64:65, :])
                    rb = rbp.tile([64, SC], F32, tag="rb", name="rb")
                    nc.sync.dma_start(out=rb[:], in_=dr[:].to_broadcast((64, SC)))
                    if hl == 0:
                        nc.vector.tensor_mul(ctxN[p][0:64, cs], cu[0:64, :], rb[:])
                    else:
                        ctmp = small.tile([64, SC], BF16, tag="ctmp", name="ctmp")
                        nc.vector.tensor_mul(ctmp[:], cu[0:64, :], rb[:])
                        nc.sync.dma_start(out=ctxN[p][64:128, cs], in_=ctmp[:])

                pending.append(norm)

        def queue_wo(sc):
            cs = slice(sc * SC, (sc + 1) * SC)
            for eb in range(E // 128):
                box = {}

                def wo_a(eb=eb, cs=cs, box=box):
                    ps = ps_wo.tile([128, SC], F32, tag="wo", name="wo_ps")
                    box["ps"] = ps
                    for kb in (0, 1):
                        nc.tensor.matmul(
                            ps[:], woT[kb][:, eb * 128:(eb + 1) * 128],
                            ctxN[kb][:, cs], start=(kb == 0), stop=False)

                def wo_b(eb=eb, cs=cs, box=box):
                    ps = box["ps"]
                    for kb in (2, 3):
                        nc.tensor.matmul(
                            ps[:], woT[kb][:, eb * 128:(eb + 1) * 128],
                            ctxN[kb][:, cs],
                            start=False, stop=(kb == NPAIR - 1))
                    osb = small.tile([128, SC], F32, tag="osb", name="osb")
                    nc.vector.tensor_copy(osb[:], ps[:])
                    nc.sync.dma_start(
                        out=out_d[eb * 128:(eb + 1) * 128, cs], in_=osb[:])

                pending.append(wo_a)
                pending.append(wo_b)

        ps_wo = None

        # warm the exp table while input DMAs run
        warm = small.tile([1, 32], F32, tag="warm", name="warm")
        nc.vector.memset(warm[:], 0.0)
        nc.scalar.activation(warm[:], warm[:], EXP)

        # pair 0: inputs + first q/k projection chunks inline (the first
        # scores need them); everything else drains during its own first
        # s-chunk, in consumer-aware order (see proj_chunks).
        p0 = proj_chunks(0)
        for chunk in p0[:3]:
            chunk()
        pending.extend(p0[3:])
        for pp in range(NPAIR):
            nc.sync.dma_start(out=woT[pp][:],
                              in_=woT_d[pp * 128:(pp + 1) * 128, :])

        for p in range(NPAIR):
            nxt = proj_chunks(p + 1) if p + 1 < NPAIR else []
            if p == NPAIR - 1:
                # last pair: V/proj pools done -> free banks for W_O
                drain(len(pending))
                proj_stack.close()
                ps_wo = ctx.enter_context(
                    tc.tile_pool(name="ps_wo", bufs=2, space="PSUM"))
            for sc in range(NSC):
                # pace next pair's projections evenly across this pair
                lo = sc * len(nxt) // NSC
                hi = (sc + 1) * len(nxt) // NSC
                pending.extend(nxt[lo:hi])
                qs = slice(sc * SC, (sc + 1) * SC)
                attn_tiles = []
                ctx_tiles = [ps_ctx.tile([65, SC], F32, tag="ctx",
                                         name=f"ctx{hl}") for hl in range(2)]

                def ctx_mm(tb, attn_tiles=attn_tiles, ctx_tiles=ctx_tiles, p=p):
                    # priority: ctx matmuls free attn slots the exps wait on
                    with tc.high_priority(offset=300):
                        for hl in range(2):
                            nc.tensor.matmul(
                                ctx_tiles[hl][:],
                                vh1[2 * p + hl][:, tb, :],
                                attn_tiles[tb][:, hl * SC:(hl + 1) * SC],
                                start=(tb == 0), stop=(tb == NTB - 1))

                for tb in range(NTB):
                    scps = ps_sc.tile([128, 2 * SC], F32, tag="sc")
                    t0 = tb * TB
                    # 4 concurrent quadrant matmuls: (row=h-half, col=t-half).
                    # High priority: scores feed the ACT bottleneck — never
                    # let drained backlog (W_O / proj) cut ahead on PE.
                    with tc.high_priority(offset=600):
                        nc.tensor.matmul(scps[0:64, 0:SC],
                                         khT[p][0:64, t0:t0 + 64],
                                         qhT[p][0:64, qs], start=True, stop=True)
                        nc.tensor.matmul(scps[64:128, 0:SC],
                                         khT[p][0:64, t0 + 64:t0 + 128],
                                         qhT[p][0:64, qs], start=True, stop=True)
                        nc.tensor.matmul(scps[0:64, SC:2 * SC],
                                         khT[p][64:128, t0:t0 + 64],
                                         qhT[p][64:128, qs], start=True, stop=True)
                        nc.tensor.matmul(scps[64:128, SC:2 * SC],
                                         khT[p][64:128, t0 + 64:t0 + 128],
                                         qhT[p][64:128, qs], start=True, stop=True)
                    at = attn_pool.tile([128, 2 * SC], BF16, tag="attn")
                    nc.scalar.activation(at[:], scps[:], EXP)
                    attn_tiles.append(at)
                    # drain BEFORE ctx_mm: pending writers (e.g. pair 0's V
                    # projections) must be emitted before their ctx readers
                    drain(2)
                    # ctx matmuls trail one t-block behind their exp
                    if tb >= 1:
                        ctx_mm(tb - 1)
                ctx_mm(NTB - 1)
                queue_norm(p, sc, ctx_tiles)
                if p == NPAIR - 1:
                    queue_wo(sc)
        drain(len(pending))


_CACHE = {}


def _build():
    if "nc" in _CACHE:
        return _CACHE["nc"]
    nc = bacc.Bacc("TRN2", target_bir_lowering=False, debug=False, num_devices=8)
    qT_d = nc.dram_tensor("qT", [8 * D, S], BF16, kind="ExternalInput").ap()
    kvT_d = nc.dram_tensor("kvT", [8 * D, S], BF16, kind="ExternalInput").ap()
    wq_d = nc.dram_tensor("wq", [8 * D, D], BF16, kind="ExternalInput").ap()
    wk_d = nc.dram_tensor("wk", [8 * D, D], BF16, kind="ExternalInput").ap()
    wv_d = nc.dram_tensor("wv", [8 * D, D], BF16, kind="ExternalInput").ap()
    woT_d = nc.dram_tensor("woT", [8 * D, E], BF16, kind="ExternalInput").ap()
    out_d = nc.dram_tensor("out", [E, S], F32, kind="ExternalOutput").ap()
    with tile.TileContext(nc) as tc:
        _emit(tc, (qT_d, kvT_d, wq_d, wk_d, wv_d, woT_d, out_d))
    nc.compile()
    _CACHE["nc"] = nc
    return nc


def _shard(query, key_value, wq, wk, wv, wo):
    """Full fp32 inputs -> list of 8 per-core input maps (bf16)."""
    in_maps = []
    for c in range(8):
        b, g = divmod(c, 2)
        gs = slice(g * 512, (g + 1) * 512)
        qT = np.ascontiguousarray(query[b][:, gs].T)
        kvT = np.ascontiguousarray(key_value[b][:, gs].T)
        # per-head [e,d] -> [d,e], stacked: rows = 64*l + d_in
        wq_p = (wq[g * 8:(g + 1) * 8] * 0.125).transpose(0, 2, 1).reshape(512, D)
        wk_p = wk[g * 8:(g + 1) * 8].transpose(0, 2, 1).reshape(512, D)
        wv_p = wv[g * 8:(g + 1) * 8].transpose(0, 2, 1).reshape(512, D)
        woT = np.ascontiguousarray(wo[:, gs].T)
        in_maps.append({
            "qT": qT.astype(bf16), "kvT": kvT.astype(bf16),
            "wq": wq_p.astype(bf16), "wk": wk_p.astype(bf16),
            "wv": wv_p.astype(bf16), "woT": woT.astype(bf16),
        })
    return in_maps


def _unshard(results, wo, bo, bv):
    bias = bo.astype(np.float64) + wo.astype(np.float64) @ bv.reshape(-1).astype(np.float64)
    outs = []
    for b in range(B):
        t = results[2 * b]["out"].astype(np.float32) + results[2 * b + 1]["out"].astype(np.float32)
        outs.append(t.T + bias.astype(np.float32))
    return np.stack(outs)


def _run(in_maps, trace=False):
    nc = _build()
    return run_bass_kernel_spmd(nc, in_maps, list(range(8)), trace=trace)


def kernel(query, key_value, wq, bq, wk, bk, wv, bv, wo, bo):
    query = np.asarray(query, np.float32)
    key_value = np.asarray(key_value, np.float32)
    wq = np.asarray(wq, np.float32)
    wk = np.asarray(wk, np.float32)
    wv = np.asarray(wv, np.float32)
    wo = np.asarray(wo, np.float32)
    bo = np.asarray(bo, np.float32)
    bv = np.asarray(bv, np.float32)
    in_maps = _shard(query, key_value, wq, wk, wv, wo)
    res = _run(in_maps, trace=False)
    return _unshard(res.results, wo, bo, bv)


# revision 36
# speedup vs baseline: 1.0057x; 1.0057x over previous
"""Multi-head cross-attention (B=4, Sq=Skv=2048, E=1024, H=16, D=64) on 8
Trainium2 NeuronCores.

Sharding: core c -> (batch b = c//2, head-group g = c%2 of 8 heads).
Each core computes, for its batch and its 8 heads:
    qh = q @ wq.T (per head), kh/vh likewise (kv),
    scoresT[t,s] = kh . qh  (transposed orientation, t on partitions),
    attnT = exp(scoresT)    (no max subtraction; scores ~ N(0,1)),
    ctxT[d,s]  = sum_t vh[t,d] attnT[t,s]       (PSUM accumulate over t),
    denom[s]   = sum_t attnT[t,s]               (65th "ones" column of vh),
    ctxN       = ctxT * (1/denom),
    outT[e,s] += woT[hd,e].T @ ctxN[hd,s]       (partial W_O, this core's heads).
Host sums the two head-group partials per batch, transposes, adds bias terms.

Bias handling: bq==0 guaranteed by the problem spec (fill=zeros); bk is
mathematically a no-op for softmax (adds a per-query constant to scores);
bv folds to +bv after normalization, handled on host via wo @ bv; bo added
on host.

dtype: bf16 on the tensor engine with fp32 PSUM accumulation.
"""

import sys
import types

import numpy as np


def _ensure_paths():
    try:
        import concourse.bass  # noqa: F401
    except ImportError:
        for p in ("/opt/trn_rl_repo", "/root/.axon_site/_ro/trn_rl_repo"):
            if p not in sys.path:
                sys.path.append(p)


def _install_ntff_hook():
    """Register the axon NTFF profiling hook if the image's antenv lacks it.
    Only needed when tracing (BASS_TRACE=1); harmless otherwise."""
    try:
        from antenv.axon_hooks import get_axon_ntff_profile_hook  # noqa: F401

        return
    except ImportError:
        pass
    try:
        import antenv
        from trn_agent_boot.trn_boot import _ntff_profile_via_ctypes

        mod = types.ModuleType("antenv.axon_hooks")
        _h = [None]
        mod.set_axon_ntff_profile_hook = lambda h: _h.__setitem__(0, h)
        mod.get_axon_ntff_profile_hook = lambda: _h[0]
        sys.modules["antenv.axon_hooks"] = mod
        antenv.axon_hooks = mod
        mod.set_axon_ntff_profile_hook(
            _ntff_profile_via_ctypes("/opt/axon/libaxon_pjrt.so")
        )
    except Exception:
        pass


_ensure_paths()
_install_ntff_hook()

import ml_dtypes  # noqa: E402
from contextlib import ExitStack  # noqa: E402

import concourse.bass as bass  # noqa: E402
import concourse.tile as tile  # noqa: E402
from concourse import bacc, mybir  # noqa: E402
from concourse.bass_utils import run_bass_kernel_spmd  # noqa: E402

BF16 = mybir.dt.bfloat16
F32 = mybir.dt.float32
bf16 = ml_dtypes.bfloat16

B, S, E, H, D = 4, 2048, 1024, 16, 64
NPAIR = 4          # head pairs per core (8 heads)
SC, NSC = 512, 4   # s-chunk
TB, NTB = 128, 16  # t-block
EXP = mybir.ActivationFunctionType.Exp


def _emit(tc, dram):
    nc = tc.nc
    qT_d, kvT_d, wq_d, wk_d, wv_d, woT_d, out_d = dram

    with ExitStack() as ctx:
        persist = ctx.enter_context(tc.tile_pool(name="persist", bufs=1))

        def ptile(shape, tag):
            return persist.tile(shape, BF16, tag=tag, name=tag)

        qhT = [ptile([128, S], f"qhT{p}") for p in range(NPAIR)]
        khT = [ptile([128, S], f"khT{p}") for p in range(NPAIR)]
        # vh1[h]: [t(128), NTB, 65]; col 64 = softmax-denominator ones column
        vh1 = [ptile([128, NTB, 65], f"vh1_{h}") for h in range(2 * NPAIR)]
        ctxN = [ptile([128, S], f"ctxN{p}") for p in range(NPAIR)]
        woT = [ptile([128, E], f"woT{p}") for p in range(NPAIR)]
        # qkv weights: one [128, NPAIR, D] tile each (one DMA submit each —
        # each dma_start costs ~650ns of Sync-engine issue time)
        wq_all = ptile([128, NPAIR, D], "wq_all")
        wk_all = ptile([128, NPAIR, D], "wk_all")
        wv_all = ptile([128, NPAIR, D], "wv_all")
        wq_sb = [wq_all[:, p, :] for p in range(NPAIR)]
        wk_sb = [wk_all[:, p, :] for p in range(NPAIR)]
        wv_sb = [wv_all[:, p, :] for p in range(NPAIR)]

        for (dst, src) in ((wq_all, wq_d), (wk_all, wk_d), (wv_all, wv_d)):
            nc.sync.dma_start(
                out=dst[:], in_=src.rearrange("(p i) e -> i p e", p=NPAIR))
        for h in range(2 * NPAIR):
            nc.vector.memset(vh1[h][:, :, 64:65], 1.0)

        inp = ctx.enter_context(tc.tile_pool(name="inp", bufs=4))
        drp = ctx.enter_context(tc.tile_pool(name="drp", bufs=4, space="DRAM"))
        attn_pool = ctx.enter_context(tc.tile_pool(name="attn", bufs=10))
        small = ctx.enter_context(tc.tile_pool(name="small", bufs=4))
        rbp = ctx.enter_context(tc.tile_pool(name="rbp", bufs=4))
        ctxu_pool = ctx.enter_context(tc.tile_pool(name="ctxu", bufs=4))
        ps_sc = ctx.enter_context(tc.tile_pool(name="ps_sc", bufs=2, space="PSUM"))
        ps_ctx = ctx.enter_context(tc.tile_pool(name="ps_ctx", bufs=2, space="PSUM"))
        proj_stack = ExitStack()
        ps_proj = proj_stack.enter_context(
            tc.tile_pool(name="ps_proj", bufs=1, space="PSUM"))
        ps_v = proj_stack.enter_context(
            tc.tile_pool(name="ps_v", bufs=1, space="PSUM"))

        # deferred PE work (ctx / W_O chunks) interleaved into later emission
        pending = []

        def drain(n):
            for _ in range(n):
                if pending:
                    pending.pop(0)()

        def proj_chunks(p):
            """Projection work for pair p as a list of small emit-callables
            (so pair p+1's projections interleave into pair p's attention)."""
            chunks = []
            state = {}

            def load_inputs():
                qT_t = inp.tile([128, S], BF16, tag="inp", name="qT_t")
                nc.sync.dma_start(out=qT_t[:], in_=qT_d[p * 128:(p + 1) * 128, :])
                kvT_t = inp.tile([128, S], BF16, tag="inp", name="kvT_t")
                nc.sync.dma_start(out=kvT_t[:], in_=kvT_d[p * 128:(p + 1) * 128, :])
                state["qT"], state["kv"] = qT_t, kvT_t

            chunks.append(load_inputs)

            def qk(which, sc):
                def go():
                    w_sb = wq_sb[p] if which == 0 else wk_sb[p]
                    src = state["qT"] if which == 0 else state["kv"]
                    dst = qhT[p] if which == 0 else khT[p]
                    ps = ps_proj.tile([128, SC], F32, tag="proj", name="ps")
                    cs = slice(sc * SC, (sc + 1) * SC)
                    nc.tensor.matmul(ps[0:64, :], w_sb[0:64, :],
                                     src[0:64, cs], start=True, stop=True)
                    nc.tensor.matmul(ps[64:128, :], w_sb[64:128, :],
                                     src[64:128, cs], start=True, stop=True)
                    nc.vector.tensor_copy(dst[:, cs], ps[:])
                return go

            def vproj(hl, tq):
                def go():
                    h = 2 * p + hl
                    hs = slice(hl * 64, (hl + 1) * 64)
                    psv = ps_v.tile([128, 4, D], F32, tag="v", name="psv")
                    for j in range(4):
                        tb = 4 * tq + j
                        nc.tensor.matmul(
                            psv[:, j, :],
                            state["kv"][hs, tb * TB:(tb + 1) * TB],
                            wv_sb[p][hs, :], start=True, stop=True)
                    nc.vector.tensor_copy(
                        vh1[h][:, 4 * tq:4 * tq + 4, 0:64], psv[:])
                return go

            # Order matters: Tile tracks deps only on already-emitted
            # instructions, so every chunk must be emitted before the first
            # instruction that reads its output. The consumer-aware order
            # below lets chunks drain lazily during the *previous* pair's
            # attention (or, for pair 0, during its own first s-chunk):
            #   q0,k0 first (first scores), then k1..k3 / v interleaved
            #   early (scores tb>=4, ctx matmuls), q1..q3 last (s-chunk>=1).
            chunks.append(qk(0, 0))
            chunks.append(qk(1, 0))
            chunks.append(vproj(0, 0))
            chunks.append(vproj(1, 0))
            chunks.append(qk(1, 1))
            chunks.append(vproj(0, 1))
            chunks.append(vproj(1, 1))
            chunks.append(qk(1, 2))
            chunks.append(vproj(0, 2))
            chunks.append(vproj(1, 2))
            chunks.append(qk(1, 3))
            chunks.append(vproj(0, 3))
            chunks.append(vproj(1, 3))
            chunks.append(qk(0, 1))
            chunks.append(qk(0, 2))
            chunks.append(qk(0, 3))
            return chunks

        def queue_norm(p, sc, ctx_tiles):
            # normalize ctx by the denominator row for both heads
            cs = slice(sc * SC, (sc + 1) * SC)
            for hl in range(2):
                ctx_ps = ctx_tiles[hl]

                def norm(p=p, hl=hl, ctx_ps=ctx_ps, cs=cs):
                    # Copy the whole [65,SC] PSUM tile to SBUF right away so
                    # the PSUM slot frees fast; the slow reciprocal then runs
                    # off the critical path.
                    cu = ctxu_pool.tile([65, SC], F32, tag="cu", name="cu")
                    nc.vector.tensor_copy(cu[:], ctx_ps[:])
                    # approx reciprocal (~3e-6 rel err). Quirks: must not be
                    # in-place, and needs a base-partition-0 range (a [64:65]
                    # slice returns garbage) — so run it over all 65 rows and
                    # use only row 64 (denominators; other rows are unused).
                    rp = small.tile([65, SC], F32, tag="r0", name="rp")
                    nc.vector.reciprocal_approx_fast(out=rp[:], in_=cu[:])
                    # partition-broadcast via DRAM bounce (SBUF sources
                    # require nonzero partition stride)
                    dr = drp.tile([1, SC], F32, tag="dr", name="dr")
                    nc.sync.dma_start(out=dr[:], in_=rp[64:65, :])
                    rb = rbp.tile([64, SC], F32, tag="rb", name="rb")
                    nc.sync.dma_start(out=rb[:], in_=dr[:].to_broadcast((64, SC)))
                    if hl == 0:
                        nc.vector.tensor_mul(ctxN[p][0:64, cs], cu[0:64, :], rb[:])
                    else:
                        ctmp = small.tile([64, SC], BF16, tag="ctmp", name="ctmp")
                        nc.vector.tensor_mul(ctmp[:], cu[0:64, :], rb[:])
                        nc.sync.dma_start(out=ctxN[p][64:128, cs], in_=ctmp[:])

                pending.append(norm)

        def queue_wo(sc):
            cs = slice(sc * SC, (sc + 1) * SC)
            for eb in range(E // 128):
                box = {}

                def wo_a(eb=eb, cs=cs, box=box):
                    ps = ps_wo.tile([128, SC], F32, tag="wo", name="wo_ps")
                    box["ps"] = ps
                    for kb in (0, 1):
                        nc.tensor.matmul(
                            ps[:], woT[kb][:, eb * 128:(eb + 1) * 128],
                            ctxN[kb][:, cs], start=(kb == 0), stop=False)

                def wo_b(eb=eb, cs=cs, box=box):
                    ps = box["ps"]
                    for kb in (2, 3):
                        nc.tensor.matmul(
                            ps[:], woT[kb][:, eb * 128:(eb + 1) * 128],
                            ctxN[kb][:, cs],
                            start=False, stop=(kb == NPAIR - 1))
                    osb = small.tile([128, SC], F32, tag="osb", name="osb")
                    nc.vector.tensor_copy(osb[:], ps[:])
                    nc.sync.dma_start(
                        out=out_d[eb * 128:(eb + 1) * 128, cs], in_=osb[:])

                pending.append(wo_a)
                pending.append(wo_b)

        ps_wo = None

        # warm the exp table while input DMAs run
        warm = small.tile([1, 32], F32, tag="warm", name="warm")
        nc.vector.memset(warm[:], 0.0)
        nc.scalar.activation(warm[:], warm[:], EXP)

        # pair 0: inputs + first q/k projection chunks inline (the first
        # scores need them); everything else drains during its own first
        # s-chunk, in consumer-aware order (see proj_chunks).
        p0 = proj_chunks(0)
        for chunk in p0[:3]:
            chunk()
        pending.extend(p0[3:])
        for pp in range(NPAIR):
            nc.sync.dma_start(out=woT[pp][:],
                              in_=woT_d[pp * 128:(pp + 1) * 128, :])

        for p in range(NPAIR):
            nxt = proj_chunks(p + 1) if p + 1 < NPAIR else []
            if p == NPAIR - 1:
                # last pair: V/proj pools done -> free banks for W_O
                drain(len(pending))
                proj_stack.close()
                ps_wo = ctx.enter_context(
                    tc.tile_pool(name="ps_wo", bufs=2, space="PSUM"))
            for sc in range(NSC):
                # pace next pair's projections evenly across this pair
                lo = sc * len(nxt) // NSC
                hi = (sc + 1) * len(nxt) // NSC
                pending.extend(nxt[lo:hi])
                qs = slice(sc * SC, (sc + 1) * SC)
                attn_tiles = []
                ctx_tiles = [ps_ctx.tile([65, SC], F32, tag="ctx",
                                         name=f"ctx{hl}") for hl in range(2)]

                def ctx_mm(tb, attn_tiles=attn_tiles, ctx_tiles=ctx_tiles, p=p):
                    for hl in range(2):
                        nc.tensor.matmul(
                            ctx_tiles[hl][:],
                            vh1[2 * p + hl][:, tb, :],
                            attn_tiles[tb][:, hl * SC:(hl + 1) * SC],
                            start=(tb == 0), stop=(tb == NTB - 1))

                for tb in range(NTB):
                    scps = ps_sc.tile([128, 2 * SC], F32, tag="sc")
                    t0 = tb * TB
                    # 4 concurrent quadrant matmuls: (row=h-half, col=t-half).
                    # High priority: scores feed the ACT bottleneck — never
                    # let drained backlog (W_O / proj) cut ahead on PE.
                    with tc.high_priority(offset=600):
                        nc.tensor.matmul(scps[0:64, 0:SC],
                                         khT[p][0:64, t0:t0 + 64],
                                         qhT[p][0:64, qs], start=True, stop=True)
                        nc.tensor.matmul(scps[64:128, 0:SC],
                                         khT[p][0:64, t0 + 64:t0 + 128],
                                         qhT[p][0:64, qs], start=True, stop=True)
                        nc.tensor.matmul(scps[0:64, SC:2 * SC],
                                         khT[p][64:128, t0:t0 + 64],
                                         qhT[p][64:128, qs], start=True, stop=True)
                        nc.tensor.matmul(scps[64:128, SC:2 * SC],
                                         khT[p][64:128, t0 + 64:t0 + 128],
                                         qhT[p][64:128, qs], start=True, stop=True)
                    at = attn_pool.tile([128, 2 * SC], BF16, tag="attn")
                    nc.scalar.activation(at[:], scps[:], EXP)
                    attn_tiles.append(at)
                    # drain BEFORE ctx_mm: pending writers (e.g. pair 0's V
                    # projections) must be emitted before their ctx readers
                    drain(2)
                    # ctx matmuls trail one t-block behind their exp
                    if tb >= 1:
                        ctx_mm(tb - 1)
                ctx_mm(NTB - 1)
                queue_norm(p, sc, ctx_tiles)
                if p == NPAIR - 1:
                    queue_wo(sc)
        drain(len(pending))


_CACHE = {}


def _build():
    if "nc" in _CACHE:
        return _CACHE["nc"]
    nc = bacc.Bacc("TRN2", target_bir_lowering=False, debug=False, num_devices=8)
    qT_d = nc.dram_tensor("qT", [8 * D, S], BF16, kind="ExternalInput").ap()
    kvT_d = nc.dram_tensor("kvT", [8 * D, S], BF16, kind="ExternalInput").ap()
    wq_d = nc.dram_tensor("wq", [8 * D, D], BF16, kind="ExternalInput").ap()
    wk_d = nc.dram_tensor("wk", [8 * D, D], BF16, kind="ExternalInput").ap()
    wv_d = nc.dram_tensor("wv", [8 * D, D], BF16, kind="ExternalInput").ap()
    woT_d = nc.dram_tensor("woT", [8 * D, E], BF16, kind="ExternalInput").ap()
    out_d = nc.dram_tensor("out", [E, S], F32, kind="ExternalOutput").ap()
    with tile.TileContext(nc) as tc:
        _emit(tc, (qT_d, kvT_d, wq_d, wk_d, wv_d, woT_d, out_d))
    nc.compile()
    _CACHE["nc"] = nc
    return nc


def _shard(query, key_value, wq, wk, wv, wo):
    """Full fp32 inputs -> list of 8 per-core input maps (bf16)."""
    in_maps = []
    for c in range(8):
        b, g = divmod(c, 2)
        gs = slice(g * 512, (g + 1) * 512)
        qT = np.ascontiguousarray(query[b][:, gs].T)
        kvT = np.ascontiguousarray(key_value[b][:, gs].T)
        # per-head [e,d] -> [d,e], stacked: rows = 64*l + d_in
        wq_p = (wq[g * 8:(g + 1) * 8] * 0.125).transpose(0, 2, 1).reshape(512, D)
        wk_p = wk[g * 8:(g + 1) * 8].transpose(0, 2, 1).reshape(512, D)
        wv_p = wv[g * 8:(g + 1) * 8].transpose(0, 2, 1).reshape(512, D)
        woT = np.ascontiguousarray(wo[:, gs].T)
        in_maps.append({
            "qT": qT.astype(bf16), "kvT": kvT.astype(bf16),
            "wq": wq_p.astype(bf16), "wk": wk_p.astype(bf16),
            "wv": wv_p.astype(bf16), "woT": woT.astype(bf16),
        })
    return in_maps


def _unshard(results, wo, bo, bv):
    bias = bo.astype(np.float64) + wo.astype(np.float64) @ bv.reshape(-1).astype(np.float64)
    outs = []
    for b in range(B):
        t = results[2 * b]["out"].astype(np.float32) + results[2 * b + 1]["out"].astype(np.float32)
        outs.append(t.T + bias.astype(np.float32))
    return np.stack(outs)


def _run(in_maps, trace=False):
    nc = _build()
    return run_bass_kernel_spmd(nc, in_maps, list(range(8)), trace=trace)


def kernel(query, key_value, wq, bq, wk, bk, wv, bv, wo, bo):
    query = np.asarray(query, np.float32)
    key_value = np.asarray(key_value, np.float32)
    wq = np.asarray(wq, np.float32)
    wk = np.asarray(wk, np.float32)
    wv = np.asarray(wv, np.float32)
    wo = np.asarray(wo, np.float32)
    bo = np.asarray(bo, np.float32)
    bv = np.asarray(bv, np.float32)
    in_maps = _shard(query, key_value, wq, wk, wv, wo)
    res = _run(in_maps, trace=False)
    return _unshard(res.results, wo, bo, bv)


# revision 37
# speedup vs baseline: 1.0277x; 1.0219x over previous
"""Multi-head cross-attention (B=4, Sq=Skv=2048, E=1024, H=16, D=64) on 8
Trainium2 NeuronCores.

Sharding: core c -> (batch b = c//2, head-group g = c%2 of 8 heads).
Each core computes, for its batch and its 8 heads:
    qh = q @ wq.T (per head), kh/vh likewise (kv),
    scoresT[t,s] = kh . qh  (transposed orientation, t on partitions),
    attnT = exp(scoresT)    (no max subtraction; scores ~ N(0,1)),
    ctxT[d,s]  = sum_t vh[t,d] attnT[t,s]       (PSUM accumulate over t),
    denom[s]   = sum_t attnT[t,s]               (65th "ones" column of vh),
    ctxN       = ctxT * (1/denom),
    outT[e,s] += woT[hd,e].T @ ctxN[hd,s]       (partial W_O, this core's heads).
Host sums the two head-group partials per batch, transposes, adds bias terms.

Bias handling: bq==0 guaranteed by the problem spec (fill=zeros); bk is
mathematically a no-op for softmax (adds a per-query constant to scores);
bv folds to +bv after normalization, handled on host via wo @ bv; bo added
on host.

dtype: bf16 on the tensor engine with fp32 PSUM accumulation.
"""

import sys
import types

import numpy as np


def _ensure_paths():
    try:
        import concourse.bass  # noqa: F401
    except ImportError:
        for p in ("/opt/trn_rl_repo", "/root/.axon_site/_ro/trn_rl_repo"):
            if p not in sys.path:
                sys.path.append(p)


def _install_ntff_hook():
    """Register the axon NTFF profiling hook if the image's antenv lacks it.
    Only needed when tracing (BASS_TRACE=1); harmless otherwise."""
    try:
        from antenv.axon_hooks import get_axon_ntff_profile_hook  # noqa: F401

        return
    except ImportError:
        pass
    try:
        import antenv
        from trn_agent_boot.trn_boot import _ntff_profile_via_ctypes

        mod = types.ModuleType("antenv.axon_hooks")
        _h = [None]
        mod.set_axon_ntff_profile_hook = lambda h: _h.__setitem__(0, h)
        mod.get_axon_ntff_profile_hook = lambda: _h[0]
        sys.modules["antenv.axon_hooks"] = mod
        antenv.axon_hooks = mod
        mod.set_axon_ntff_profile_hook(
            _ntff_profile_via_ctypes("/opt/axon/libaxon_pjrt.so")
        )
    except Exception:
        pass


_ensure_paths()
_install_ntff_hook()

import ml_dtypes  # noqa: E402
from contextlib import ExitStack  # noqa: E402

import concourse.bass as bass  # noqa: E402
import concourse.tile as tile  # noqa: E402
from concourse import bacc, mybir  # noqa: E402
from concourse.bass_utils import run_bass_kernel_spmd  # noqa: E402

BF16 = mybir.dt.bfloat16
F32 = mybir.dt.float32
bf16 = ml_dtypes.bfloat16

B, S, E, H, D = 4, 2048, 1024, 16, 64
NPAIR = 4          # head pairs per core (8 heads)
SC, NSC = 512, 4   # s-chunk
TB, NTB = 128, 16  # t-block
EXP = mybir.ActivationFunctionType.Exp


def _emit(tc, dram):
    nc = tc.nc
    qT_d, kvT_d, wq_d, wk_d, wv_d, woT_d, out_d = dram

    with ExitStack() as ctx:
        persist = ctx.enter_context(tc.tile_pool(name="persist", bufs=1))

        def ptile(shape, tag):
            return persist.tile(shape, BF16, tag=tag, name=tag)

        qhT = [ptile([128, S], f"qhT{p}") for p in range(NPAIR)]
        khT = [ptile([128, S], f"khT{p}") for p in range(NPAIR)]
        # vh1[h]: [t(128), NTB, 65]; col 64 = softmax-denominator ones column
        vh1 = [ptile([128, NTB, 65], f"vh1_{h}") for h in range(2 * NPAIR)]
        ctxN = [ptile([128, S], f"ctxN{p}") for p in range(NPAIR)]
        woT = [ptile([128, E], f"woT{p}") for p in range(NPAIR)]
        # qkv weights: one [128, NPAIR, D] tile each (one DMA submit each —
        # each dma_start costs ~650ns of Sync-engine issue time)
        wq_all = ptile([128, NPAIR, D], "wq_all")
        wk_all = ptile([128, NPAIR, D], "wk_all")
        wv_all = ptile([128, NPAIR, D], "wv_all")
        wq_sb = [wq_all[:, p, :] for p in range(NPAIR)]
        wk_sb = [wk_all[:, p, :] for p in range(NPAIR)]
        wv_sb = [wv_all[:, p, :] for p in range(NPAIR)]

        for (dst, src) in ((wq_all, wq_d), (wk_all, wk_d), (wv_all, wv_d)):
            nc.sync.dma_start(
                out=dst[:], in_=src.rearrange("(p i) e -> i p e", p=NPAIR))
        for h in range(2 * NPAIR):
            nc.vector.memset(vh1[h][:, :, 64:65], 1.0)

        inp = ctx.enter_context(tc.tile_pool(name="inp", bufs=4))
        drp = ctx.enter_context(tc.tile_pool(name="drp", bufs=4, space="DRAM"))
        attn_pool = ctx.enter_context(tc.tile_pool(name="attn", bufs=6))
        small = ctx.enter_context(tc.tile_pool(name="small", bufs=4))
        rbp = ctx.enter_context(tc.tile_pool(name="rbp", bufs=4))
        ctxu_pool = ctx.enter_context(tc.tile_pool(name="ctxu", bufs=4))
        ps_sc = ctx.enter_context(tc.tile_pool(name="ps_sc", bufs=2, space="PSUM"))
        ps_ctx = ctx.enter_context(tc.tile_pool(name="ps_ctx", bufs=2, space="PSUM"))
        proj_stack = ExitStack()
        ps_proj = proj_stack.enter_context(
            tc.tile_pool(name="ps_proj", bufs=1, space="PSUM"))
        ps_v = proj_stack.enter_context(
            tc.tile_pool(name="ps_v", bufs=1, space="PSUM"))

        # deferred PE work (ctx / W_O chunks) interleaved into later emission
        pending = []

        def drain(n):
            for _ in range(n):
                if pending:
                    pending.pop(0)()

        def proj_chunks(p):
            """Projection work for pair p as a list of small emit-callables
            (so pair p+1's projections interleave into pair p's attention)."""
            chunks = []
            state = {}

            def load_inputs():
                qT_t = inp.tile([128, S], BF16, tag="inp", name="qT_t")
                nc.sync.dma_start(out=qT_t[:], in_=qT_d[p * 128:(p + 1) * 128, :])
                kvT_t = inp.tile([128, S], BF16, tag="inp", name="kvT_t")
                nc.sync.dma_start(out=kvT_t[:], in_=kvT_d[p * 128:(p + 1) * 128, :])
                state["qT"], state["kv"] = qT_t, kvT_t

            chunks.append(load_inputs)

            def qk(which, sc):
                def go():
                    w_sb = wq_sb[p] if which == 0 else wk_sb[p]
                    src = state["qT"] if which == 0 else state["kv"]
                    dst = qhT[p] if which == 0 else khT[p]
                    ps = ps_proj.tile([128, SC], F32, tag="proj", name="ps")
                    cs = slice(sc * SC, (sc + 1) * SC)
                    nc.tensor.matmul(ps[0:64, :], w_sb[0:64, :],
                                     src[0:64, cs], start=True, stop=True)
                    nc.tensor.matmul(ps[64:128, :], w_sb[64:128, :],
                                     src[64:128, cs], start=True, stop=True)
                    nc.vector.tensor_copy(dst[:, cs], ps[:])
                return go

            def vproj(hl, tq):
                def go():
                    h = 2 * p + hl
                    hs = slice(hl * 64, (hl + 1) * 64)
                    psv = ps_v.tile([128, 4, D], F32, tag="v", name="psv")
                    for j in range(4):
                        tb = 4 * tq + j
                        nc.tensor.matmul(
                            psv[:, j, :],
                            state["kv"][hs, tb * TB:(tb + 1) * TB],
                            wv_sb[p][hs, :], start=True, stop=True)
                    nc.vector.tensor_copy(
                        vh1[h][:, 4 * tq:4 * tq + 4, 0:64], psv[:])
                return go

            # Order matters: Tile tracks deps only on already-emitted
            # instructions, so every chunk must be emitted before the first
            # instruction that reads its output. The consumer-aware order
            # below lets chunks drain lazily during the *previous* pair's
            # attention (or, for pair 0, during its own first s-chunk):
            #   q0,k0 first (first scores), then k1..k3 / v interleaved
            #   early (scores tb>=4, ctx matmuls), q1..q3 last (s-chunk>=1).
            chunks.append(qk(0, 0))
            chunks.append(qk(1, 0))
            chunks.append(vproj(0, 0))
            chunks.append(vproj(1, 0))
            chunks.append(qk(1, 1))
            chunks.append(vproj(0, 1))
            chunks.append(vproj(1, 1))
            chunks.append(qk(1, 2))
            chunks.append(vproj(0, 2))
            chunks.append(vproj(1, 2))
            chunks.append(qk(1, 3))
            chunks.append(vproj(0, 3))
            chunks.append(vproj(1, 3))
            chunks.append(qk(0, 1))
            chunks.append(qk(0, 2))
            chunks.append(qk(0, 3))
            return chunks

        def queue_norm(p, sc, ctx_tiles):
            # normalize ctx by the denominator row for both heads
            cs = slice(sc * SC, (sc + 1) * SC)
            for hl in range(2):
                ctx_ps = ctx_tiles[hl]

                def norm(p=p, hl=hl, ctx_ps=ctx_ps, cs=cs):
                    # Copy the whole [65,SC] PSUM tile to SBUF right away so
                    # the PSUM slot frees fast; the slow reciprocal then runs
                    # off the critical path.
                    cu = ctxu_pool.tile([65, SC], F32, tag="cu", name="cu")
                    nc.vector.tensor_copy(cu[:], ctx_ps[:])
                    # approx reciprocal (~3e-6 rel err). Quirks: must not be
                    # in-place, and needs a base-partition-0 range (a [64:65]
                    # slice returns garbage) — so run it over all 65 rows and
                    # use only row 64 (denominators; other rows are unused).
                    rp = small.tile([65, SC], F32, tag="r0", name="rp")
                    nc.vector.reciprocal_approx_fast(out=rp[:], in_=cu[:])
                    # partition-broadcast via DRAM bounce (SBUF sources
                    # require nonzero partition stride)
                    dr = drp.tile([1, SC], F32, tag="dr", name="dr")
                    nc.sync.dma_start(out=dr[:], in_=rp[64:65, :])
                    rb = rbp.tile([64, SC], F32, tag="rb", name="rb")
                    nc.sync.dma_start(out=rb[:], in_=dr[:].to_broadcast((64, SC)))
                    if hl == 0:
                        nc.vector.tensor_mul(ctxN[p][0:64, cs], cu[0:64, :], rb[:])
                    else:
                        ctmp = small.tile([64, SC], BF16, tag="ctmp", name="ctmp")
                        nc.vector.tensor_mul(ctmp[:], cu[0:64, :], rb[:])
                        nc.sync.dma_start(out=ctxN[p][64:128, cs], in_=ctmp[:])

                pending.append(norm)

        def queue_wo(sc):
            cs = slice(sc * SC, (sc + 1) * SC)
            for eb in range(E // 128):
                box = {}

                def wo_a(eb=eb, cs=cs, box=box):
                    ps = ps_wo.tile([128, SC], F32, tag="wo", name="wo_ps")
                    box["ps"] = ps
                    for kb in (0, 1):
                        nc.tensor.matmul(
                            ps[:], woT[kb][:, eb * 128:(eb + 1) * 128],
                            ctxN[kb][:, cs], start=(kb == 0), stop=False)

                def wo_b(eb=eb, cs=cs, box=box):
                    ps = box["ps"]
                    for kb in (2, 3):
                        nc.tensor.matmul(
                            ps[:], woT[kb][:, eb * 128:(eb + 1) * 128],
                            ctxN[kb][:, cs],
                            start=False, stop=(kb == NPAIR - 1))
                    osb = small.tile([128, SC], F32, tag="osb", name="osb")
                    nc.vector.tensor_copy(osb[:], ps[:])
                    nc.sync.dma_start(
                        out=out_d[eb * 128:(eb + 1) * 128, cs], in_=osb[:])

                pending.append(wo_a)
                pending.append(wo_b)

        ps_wo = None

        # warm the exp table while input DMAs run
        warm = small.tile([1, 32], F32, tag="warm", name="warm")
        nc.vector.memset(warm[:], 0.0)
        nc.scalar.activation(warm[:], warm[:], EXP)

        # pair 0: inputs + first q/k projection chunks inline (the first
        # scores need them); everything else drains during its own first
        # s-chunk, in consumer-aware order (see proj_chunks).
        p0 = proj_chunks(0)
        for chunk in p0[:3]:
            chunk()
        pending.extend(p0[3:])
        for pp in range(NPAIR):
            nc.sync.dma_start(out=woT[pp][:],
                              in_=woT_d[pp * 128:(pp + 1) * 128, :])

        for p in range(NPAIR):
            nxt = proj_chunks(p + 1) if p + 1 < NPAIR else []
            if p == NPAIR - 1:
                # last pair: V/proj pools done -> free banks for W_O
                drain(len(pending))
                proj_stack.close()
                ps_wo = ctx.enter_context(
                    tc.tile_pool(name="ps_wo", bufs=2, space="PSUM"))
            for sc in range(NSC):
                # pace next pair's projections evenly across this pair
                lo = sc * len(nxt) // NSC
                hi = (sc + 1) * len(nxt) // NSC
                pending.extend(nxt[lo:hi])
                qs = slice(sc * SC, (sc + 1) * SC)
                attn_tiles = []
                ctx_tiles = [ps_ctx.tile([65, SC], F32, tag="ctx",
                                         name=f"ctx{hl}") for hl in range(2)]

                def ctx_mm(tb, attn_tiles=attn_tiles, ctx_tiles=ctx_tiles, p=p):
                    for hl in range(2):
                        nc.tensor.matmul(
                            ctx_tiles[hl][:],
                            vh1[2 * p + hl][:, tb, :],
                            attn_tiles[tb][:, hl * SC:(hl + 1) * SC],
                            start=(tb == 0), stop=(tb == NTB - 1))

                for tb in range(NTB):
                    scps = ps_sc.tile([128, 2 * SC], F32, tag="sc")
                    t0 = tb * TB
                    # 4 concurrent quadrant matmuls: (row=h-half, col=t-half).
                    # High priority: scores feed the ACT bottleneck — never
                    # let drained backlog (W_O / proj) cut ahead on PE.
                    with tc.high_priority(offset=600):
                        nc.tensor.matmul(scps[0:64, 0:SC],
                                         khT[p][0:64, t0:t0 + 64],
                                         qhT[p][0:64, qs], start=True, stop=True)
                        nc.tensor.matmul(scps[64:128, 0:SC],
                                         khT[p][0:64, t0 + 64:t0 + 128],
                                         qhT[p][0:64, qs], start=True, stop=True)
                        nc.tensor.matmul(scps[0:64, SC:2 * SC],
                                         khT[p][64:128, t0:t0 + 64],
                                         qhT[p][64:128, qs], start=True, stop=True)
                        nc.tensor.matmul(scps[64:128, SC:2 * SC],
                                         khT[p][64:128, t0 + 64:t0 + 128],
                                         qhT[p][64:128, qs], start=True, stop=True)
                    at = attn_pool.tile([128, 2 * SC], BF16, tag="attn")
                    nc.scalar.activation(at[:], scps[:], EXP)
                    attn_tiles.append(at)
                    # drain BEFORE ctx_mm: pending writers (e.g. pair 0's V
                    # projections) must be emitted before their ctx readers
                    drain(2)
                    # ctx matmuls trail one t-block behind their exp
                    if tb >= 1:
                        ctx_mm(tb - 1)
                ctx_mm(NTB - 1)
                queue_norm(p, sc, ctx_tiles)
                if p == NPAIR - 1:
                    queue_wo(sc)
        drain(len(pending))


_CACHE = {}


def _build():
    if "nc" in _CACHE:
        return _CACHE["nc"]
    nc = bacc.Bacc("TRN2", target_bir_lowering=False, debug=False, num_devices=8)
    qT_d = nc.dram_tensor("qT", [8 * D, S], BF16, kind="ExternalInput").ap()
    kvT_d = nc.dram_tensor("kvT", [8 * D, S], BF16, kind="ExternalInput").ap()
    wq_d = nc.dram_tensor("wq", [8 * D, D], BF16, kind="ExternalInput").ap()
    wk_d = nc.dram_tensor("wk", [8 * D, D], BF16, kind="ExternalInput").ap()
    wv_d = nc.dram_tensor("wv", [8 * D, D], BF16, kind="ExternalInput").ap()
    woT_d = nc.dram_tensor("woT", [8 * D, E], BF16, kind="ExternalInput").ap()
    out_d = nc.dram_tensor("out", [E, S], F32, kind="ExternalOutput").ap()
    with tile.TileContext(nc) as tc:
        _emit(tc, (qT_d, kvT_d, wq_d, wk_d, wv_d, woT_d, out_d))
    nc.compile()
    _CACHE["nc"] = nc
    return nc


def _shard(query, key_value, wq, wk, wv, wo):
    """Full fp32 inputs -> list of 8 per-core input maps (bf16)."""
    in_maps = []
    for c in range(8):
        b, g = divmod(c, 2)
        gs = slice(g * 512, (g + 1) * 512)
        qT = np.ascontiguousarray(query[b][:, gs].T)
        kvT = np.ascontiguousarray(key_value[b][:, gs].T)
        # per-head [e,d] -> [d,e], stacked: rows = 64*l + d_in
        wq_p = (wq[g * 8:(g + 1) * 8] * 0.125).transpose(0, 2, 1).reshape(512, D)
        wk_p = wk[g * 8:(g + 1) * 8].transpose(0, 2, 1).reshape(512, D)
        wv_p = wv[g * 8:(g + 1) * 8].transpose(0, 2, 1).reshape(512, D)
        woT = np.ascontiguousarray(wo[:, gs].T)
        in_maps.append({
            "qT": qT.astype(bf16), "kvT": kvT.astype(bf16),
            "wq": wq_p.astype(bf16), "wk": wk_p.astype(bf16),
            "wv": wv_p.astype(bf16), "woT": woT.astype(bf16),
        })
    return in_maps


def _unshard(results, wo, bo, bv):
    bias = bo.astype(np.float64) + wo.astype(np.float64) @ bv.reshape(-1).astype(np.float64)
    outs = []
    for b in range(B):
        t = results[2 * b]["out"].astype(np.float32) + results[2 * b + 1]["out"].astype(np.float32)
        outs.append(t.T + bias.astype(np.float32))
    return np.stack(outs)


def _run(in_maps, trace=False):
    nc = _build()
    return run_bass_kernel_spmd(nc, in_maps, list(range(8)), trace=trace)


def kernel(query, key_value, wq, bq, wk, bk, wv, bv, wo, bo):
    query = np.asarray(query, np.float32)
    key_value = np.asarray(key_value, np.float32)
    wq = np.asarray(wq, np.float32)
    wk = np.asarray(wk, np.float32)
    wv = np.asarray(wv, np.float32)
    wo = np.asarray(wo, np.float32)
    bo = np.asarray(bo, np.float32)
    bv = np.asarray(bv, np.float32)
    in_maps = _shard(query, key_value, wq, wk, wv, wo)
    res = _run(in_maps, trace=False)
    return _unshard(res.results, wo, bo, bv)
